# revision 22
# baseline (speedup 1.0000x reference)
"""Tensor-parallel GQA attention layer for one TRN2 chip (8 NeuronCores).

Problem (hardcoded): x [1, 2048, 2048] f32, w_qkv [3072, 2048] f32,
w_o [2048, 2048] f32; NH=32 q heads, KVH=8 kv heads, D=64, causal, RoPE
(non-interleaved half rotation), GQA group = 4.

Sharding: core c owns kv head c and q heads [4c, 4c+4). Each core:
  1. qkv^T projection in bf16 (fp8 here fails the 2e-2 error budget),
     x^T/w_qkv staged host-side as [128, kc, *] chunks, fed from two DMA
     queues
  2. RoPE on q/k (DVE), outputs fp8e4 in per-seq-half [32, 2, 1024]
     DoubleRow layout
  3. causal flash attention, head-pair loop: scores via fp8 DoubleRow
     (D=64 as 2x32), shared K/V weight loads across the pair, one exp per
     (kb, qc) covering both heads, V'=[V|ones] accumulates the softmax
     denominator; sender-side normalization: packed [128,8] reciprocal +
     gpsimd partition_broadcast (keeps PE/PSUM out of the chain)
  4. AllToAll per head pair (normalized bf16 payload); each group is
     unpacked by a single transposed DMA into its own tile so the o-proj
     dependency is exact (multi-writer unpacking raced)
  5. o^T projection, g-major accumulation: the first group's 64 matmuls
     run at full clock while the second AllToAll is in flight
Host concatenates the 8 [256, 2048] f32 outputs.
"""

import math

import numpy as np
import ml_dtypes

import concourse.bass as bass
import concourse.bacc as bacc
import concourse.tile as tile
import concourse.mybir as mybir
from concourse import bass_utils
from concourse.tile_legalize import tile_legalize as _tile_legalize_orig

N_CORES = 8
S = 2048           # sequence length
H = 2048           # hidden size
NH, KVH, D = 32, 8, 64
QH = NH // N_CORES # q heads per core = 4
F = QH * D + 2 * D # w_qkv rows per core = 384
SB = S // N_CORES  # seq rows per core = 256
KC = H // 128      # 16 contraction chunks of 128
KC2 = KC // 2      # 8 DoubleRow chunks of 256

BF16 = mybir.dt.bfloat16
FP8 = mybir.dt.float8e4
F32 = mybir.dt.float32
DR = mybir.MatmulPerfMode.DoubleRow
SCALE = 1.0 / math.sqrt(D)

_NC_CACHE = {}

# PE-stream instruction types that do not disturb loaded PE weights (they
# either run on other engines or are waits). Anything else resets the
# "weights currently loaded" tracking.
_LDW_SAFE = {
    "InstMatmult", "InstEventSemaphore", "InstTensorCopy", "InstTensorTensor",
    "InstActivation", "InstDMACopy", "InstReciprocal", "InstMemset",
    "InstCollectiveCompute", "InstTensorScalarPtr", "InstCustomDveAnt",
    "InstCopyPredicated", "InstTensorReduce", "InstIota",
}


def _tile_legalize_dedup_ldw(ordered_instructions_by_block, nc):
    """tile_legalize, then drop InstLdweights that reload the exact weights
    already resident in the PE array (consecutive duplicates with only
    non-PE-array-disturbing instructions in between). All matmul weights in
    this kernel are write-once tiles, so a duplicate load is always
    redundant."""
    out = _tile_legalize_orig(ordered_instructions_by_block, nc)
    for bb, insts in out.items():
        last_key = None
        kept = []
        for ins in insts:
            tn = type(ins).__name__
            if tn == "InstLdweights":
                key = repr(ins.ins[0])
                if key == last_key:
                    continue
                last_key = key
            elif tn in _LDW_SAFE:
                pass
            else:
                last_key = None
            kept.append(ins)
        out[bb] = kept
    return out


if getattr(tile.tile_legalize, "__name__", "") != "_tile_legalize_dedup_ldw":
    tile.tile_legalize = _tile_legalize_dedup_ldw


def _build_nc():
    nc = bacc.Bacc("TRN2", target_bir_lowering=False, debug=False,
                   num_devices=N_CORES)

    xt = nc.dram_tensor("xt", [128, KC, S], BF16, kind="ExternalInput")
    wq = nc.dram_tensor("wq", [128, KC, F], BF16, kind="ExternalInput")
    wo = nc.dram_tensor("wo", [H, H], BF16, kind="ExternalInput")
    cos4 = nc.dram_tensor("cos4", [128, S], BF16, kind="ExternalInput")
    sin4 = nc.dram_tensor("sin4", [128, S], BF16, kind="ExternalInput")
    tri = nc.dram_tensor("tri", [128, 128], BF16, kind="ExternalInput")
    ident = nc.dram_tensor("ident", [128, 128], BF16, kind="ExternalInput")
    out = nc.dram_tensor("out", [SB, H], F32, kind="ExternalOutput")

    with tile.TileContext(nc) as tc:
        with (
            tc.tile_pool(name="const", bufs=1) as const_pool,
            tc.tile_pool(name="persist", bufs=1) as persist,
            tc.tile_pool(name="pt_pool", bufs=16) as pt_pool,
            tc.tile_pool(name="dram", bufs=1, space="DRAM") as dram,
        ):
            # ---- persistent tiles ----
            # per-pair, per-seq-half Q^T (fp8, DoubleRow [32, 2(j), 2(h),
            # 1024]) so one DR matmul produces both heads' scores; K^T per
            # seq-half
            qt_sb = [[persist.tile([32, 2, 2, 1024], FP8,
                                   name=f"qtp{hp}_{hf}")
                      for hf in range(2)] for hp in range(2)]
            kt_sb = [persist.tile([32, 2, 1024], FP8, name=f"kt{hf}")
                     for hf in range(2)]
            vp_sb = persist.tile([128, KC, D + 1], BF16)
            # per-a2a-group o-proj inputs, each written by exactly ONE DMA:
            # att_g[g][:, i, :] = chunk kc=2i+g (heads 4i+2g, 4i+2g+1)
            att_g = [persist.tile([128, N_CORES, SB], BF16, name=f"attg{g}")
                     for g in range(2)]

            tri_sb = const_pool.tile([128, 128], BF16)
            id_sb = const_pool.tile([128, 128], BF16)
            cos_sb = const_pool.tile([128, S], BF16)
            sin_sb = const_pool.tile([128, S], BF16)

            def emit_s_exp(hp, qc, kb, sc_pool):
                """Scores (fp8 DoubleRow, both heads of the pair into one
                PSUM tile) + causal bias + one exp. Returns (pt, s0, kb)."""
                q0 = qc * 512
                k0 = kb * 128
                s0 = max(0, k0 - q0)
                sc = sc_pool.tile([128, 2, 512], F32, name="sc", tag="sc")
                khf, kof = kb // 8, k0 % 1024
                qhf, qof = qc // 2, (q0 % 1024) + s0
                kslc = kt_sb[khf][:, :, kof:kof + 128]
                for hh in range(2):
                    nc.tensor.matmul(
                        sc[:, hh, s0:512], kslc,
                        qt_sb[hp][qhf][:, :, hh, qof:qof + 512 - s0],
                        start=True, stop=True, perf_mode=DR)
                if kb >= 4 * qc:  # diagonal block: causal bias
                    for hh in range(2):
                        nc.tensor.matmul(
                            sc[:, hh, s0:s0 + 128], tri_sb[:], id_sb[:],
                            start=False, stop=True, skip_group_check=True)
                pt = pt_pool.tile([128, 2, 512], BF16, name="pt", tag="pt")
                nc.scalar.activation(
                    pt[:, :, s0:512], sc[:, :, s0:512],
                    mybir.ActivationFunctionType.Exp, scale=SCALE)
                return (pt, s0, kb)

            prestash = {}

            # ---- phase 0/1: input DMA issue + qkv^T projection + RoPE ----
            with (
                tc.tile_pool(name="xt_pool", bufs=1) as xt_pool,
                tc.tile_pool(name="qkvh_pool", bufs=1) as qkvh_pool,
                tc.tile_pool(name="rope_tmp", bufs=1) as rope_tmp,
                tc.tile_pool(name="pp_ps", bufs=1, space="PSUM") as pp_ps,
                tc.tile_pool(name="tp_ps", bufs=2, space="PSUM") as tp_ps,
            ):
                xt_sb = xt_pool.tile([128, KC, S], BF16)
                wq_sb = xt_pool.tile([128, KC, F], BF16)
                nc.sync.dma_start(wq_sb[:, 0:1, :], wq[:, 0:1, :])
                nc.sync.dma_start(xt_sb[:, 0, 0:512], xt[:, 0, 0:512])
                nc.sync.dma_start(xt_sb[:, 0, 512:S], xt[:, 0, 512:S])
                nc.scalar.dma_start(xt_sb[:, 1:2, :], xt[:, 1:2, :])
                nc.sync.dma_start(wq_sb[:, 1:KC, :], wq[:, 1:KC, :])
                for kc in range(2, KC):
                    eng = nc.sync if kc % 2 == 0 else nc.scalar
                    eng.dma_start(xt_sb[:, kc:kc + 1, :], xt[:, kc:kc + 1, :])
                    if kc == 5:
                        nc.scalar.dma_start(cos_sb[:], cos4[:])
                        nc.sync.dma_start(sin_sb[:], sin4[:])
                nc.sync.dma_start(tri_sb[:], tri[:])
                nc.sync.dma_start(id_sb[:], ident[:])
                nc.gpsimd.memset(vp_sb[:, :, D:D + 1], 1.0)
                # preload the Exp activation table while the PE works
                warm = rope_tmp.tile([1, 1], BF16, name="warm",
                                     padded_shape=[128, 1])
                nc.gpsimd.memset(warm[:], 0.0)
                nc.scalar.activation(warm[:], warm[:],
                                     mybir.ActivationFunctionType.Exp)

                def emit_qkv_m(half, m, qkvh):
                    pp = [pp_ps.tile([128, 512], F32, name=f"pp{half}{m}{nq}",
                                     tag="pp", bufs=6) for nq in range(2)]
                    for kc in range(KC):
                        lhsT = wq_sb[:, kc, m * 128:(m + 1) * 128]
                        for nq in range(2):
                            c0 = half * 1024 + nq * 512
                            nc.tensor.matmul(
                                pp[nq][:], lhsT,
                                xt_sb[:, kc, c0:c0 + 512],
                                start=(kc == 0), stop=(kc == KC - 1))
                    for nq in range(2):
                        dst = qkvh[:, m, nq * 512:(nq + 1) * 512]
                        if m == 2:
                            nc.vector.tensor_copy(dst, pp[nq][:])
                        else:
                            nc.scalar.activation(
                                dst, pp[nq][:],
                                mybir.ActivationFunctionType.Copy)

                def emit_kv_extras(half, qkvh):
                    hs = slice(half * 1024, (half + 1) * 1024)
                    k1 = qkvh[0:32, 2, :]
                    k2_t = rope_tmp.tile([32, 1024], BF16,
                                         name=f"k2_{half}", tag="k2",
                                         bufs=2, padded_shape=[128, 1024])
                    nc.gpsimd.dma_start(k2_t[:], qkvh[32:64, 2, :])
                    k2 = k2_t[:]
                    u1 = rope_tmp.tile([32, 1024], BF16, name="u1",
                                       tag="kt", bufs=2,
                                       padded_shape=[128, 1024])
                    u2 = rope_tmp.tile([32, 1024], BF16, name="u2",
                                       tag="kt", bufs=2,
                                       padded_shape=[128, 1024])
                    kA = rope_tmp.tile([32, 1024], FP8, name="kA",
                                       tag="kf", bufs=4,
                                       padded_shape=[128, 1024])
                    kB = rope_tmp.tile([32, 1024], FP8, name="kB",
                                       tag="kf", bufs=4,
                                       padded_shape=[128, 1024])
                    c32 = cos_sb[0:32, hs]
                    s32 = sin_sb[0:32, hs]
                    with nc.allow_low_precision(reason="fp8 attention"):
                        nc.vector.tensor_mul(u1[:], k1, c32)
                        nc.vector.tensor_mul(u2[:], k2, s32)
                        nc.vector.tensor_sub(kA[:], u1[:], u2[:])
                        nc.vector.tensor_mul(u1[:], k2, c32)
                        nc.vector.tensor_mul(u2[:], k1, s32)
                        nc.vector.tensor_add(kB[:], u1[:], u2[:])
                    nc.sync.dma_start(kt_sb[half][:, 0, :], kA[:])
                    nc.gpsimd.dma_start(kt_sb[half][:, 1, :], kB[:])
                    vt_bf = rope_tmp.tile([64, 1024], BF16,
                                          name=f"vt{half}", tag="vt",
                                          bufs=2, padded_shape=[128, 1024])
                    nc.gpsimd.dma_start(vt_bf[:], qkvh[64:128, 2, :])
                    for kb in range(half * 8, half * 8 + 8):
                        tp = tp_ps.tile([128, D], BF16, name="tp")
                        nc.tensor.transpose(
                            tp[:],
                            vt_bf[:, (kb % 8) * 128:(kb % 8) * 128 + 128],
                            id_sb[0:64, 0:64])
                        nc.vector.tensor_copy(vp_sb[:, kb, 0:D], tp[:])

                def emit_qrope(half, qkvh):
                    hs = slice(half * 1024, (half + 1) * 1024)
                    q1 = qkvh[:, 0, :]
                    q2 = qkvh[:, 1, :]
                    t1 = rope_tmp.tile([128, 1024], BF16, name="t1", tag="qt",
                                       bufs=2)
                    t2 = rope_tmp.tile([128, 1024], BF16, name="t2", tag="qt",
                                       bufs=2)
                    qA = rope_tmp.tile([128, 1024], FP8, name="qA", tag="qf",
                                       bufs=4)
                    qB = rope_tmp.tile([128, 1024], FP8, name="qB", tag="qf",
                                       bufs=4)
                    with nc.allow_low_precision(reason="fp8 attention"):
                        nc.vector.tensor_mul(t1[:], q1, cos_sb[:, hs])
                        nc.vector.tensor_mul(t2[:], q2, sin_sb[:, hs])
                        nc.vector.tensor_sub(qA[:], t1[:], t2[:])
                        nc.vector.tensor_mul(t1[:], q2, cos_sb[:, hs])
                        nc.vector.tensor_mul(t2[:], q1, sin_sb[:, hs])
                        nc.vector.tensor_add(qB[:], t1[:], t2[:])
                    for h in range(QH):
                        sl = slice(h * 32, (h + 1) * 32)
                        hp, hh = h // 2, h % 2
                        nc.sync.dma_start(qt_sb[hp][half][:, 0, hh, :],
                                          qA[sl, :])
                        nc.gpsimd.dma_start(qt_sb[hp][half][:, 1, hh, :],
                                            qB[sl, :])

                qkvh0 = qkvh_pool.tile([128, 3, 1024], BF16, name="qkvh0",
                                       tag="qkvh", bufs=2)
                emit_qkv_m(0, 2, qkvh0)
                emit_kv_extras(0, qkvh0)
                emit_qkv_m(0, 0, qkvh0)
                emit_qkv_m(0, 1, qkvh0)
                emit_qrope(0, qkvh0)

                # half 1 interleaved with prestashed scores+exp of pair 0,
                # q-chunks 0/1 (they depend only on half-0 outputs); their
                # AVs run in the attention phase when PSUM frees up.
                qkvh1 = qkvh_pool.tile([128, 3, 1024], BF16, name="qkvh1",
                                       tag="qkvh", bufs=2)
                emit_qkv_m(1, 2, qkvh1)
                emit_kv_extras(1, qkvh1)
                emit_qkv_m(1, 0, qkvh1)
                emit_qkv_m(1, 1, qkvh1)
                emit_qrope(1, qkvh1)

            # wo occupies the SBUF space freed by xt; its 8 MB DMA streams
            # during attention and must land before phase 3.
            wo_ctx = tc.tile_pool(name="wo_pool", bufs=1)
            wo_pool = wo_ctx.__enter__()
            wo_sb = wo_pool.tile([128, KC, H], BF16)
            nc.gpsimd.dma_start(
                wo_sb[:], wo[:].rearrange("(kc p) f -> p kc f", p=128))

            # ---- phase 2: attention (head pairs) ----
            # a2a group g carries local heads {2g, 2g+1}; payload is the
            # normalized bf16 attention output, rows h*64..h*64+64.
            a2a_in = [dram.tile([N_CORES, 128, SB], BF16, name=f"a2ai{g}")
                      for g in range(2)]
            a2a_out = [dram.tile([N_CORES, 128, SB], BF16, name=f"a2ao{g}")
                       for g in range(2)]

            def emit_a2a(g):
                nc.gpsimd.collective_compute(
                    "AllToAll",
                    mybir.AluOpType.bypass,
                    replica_groups=[list(range(N_CORES))],
                    ins=[a2a_in[g][:]],
                    outs=[a2a_out[g][:]],
                )

            def unpack(g):
                nc.sync.dma_start(att_g[g][:],
                                  a2a_out[g][:].transpose([1, 0, 2]))

            with (
                tc.tile_pool(name="sc_ps", bufs=2, space="PSUM") as sc_ps,
                tc.tile_pool(name="acc_ps", bufs=4, space="PSUM") as acc_ps,
                tc.tile_pool(name="nrm_pool", bufs=1) as nrm_pool,
            ):
                for hp in range(2):
                    for qc in range(4):
                        acc = [acc_ps.tile([D + 1, 512], F32,
                                           name=f"acc{hp}{qc}{hh}", tag="acc")
                               for hh in range(2)]
                        kbmax = 4 * qc + 4
                        pend = None  # (pt, s0, kb) awaiting AV emission
                        for kb in range(kbmax):
                            if (hp, qc, kb) in prestash:
                                nxt = prestash[(hp, qc, kb)]
                            else:
                                nxt = emit_s_exp(hp, qc, kb, sc_ps)
                            if pend is not None:
                                _emit_av(nc, acc, vp_sb, pend, kbmax)
                            pend = nxt
                        _emit_av(nc, acc, vp_sb, pend, kbmax)

                        # ---- sender-side normalize + ship ----
                        rawA = [nrm_pool.tile([D + 1, 512], BF16,
                                              name=f"rawA{hh}", tag="rawA",
                                              bufs=4, padded_shape=[128, 512])
                                for hh in range(2)]
                        for hh in range(2):
                            nc.vector.tensor_copy(rawA[hh][:], acc[hh][:])
                        dpk = nrm_pool.tile([128, 8], BF16, name="dpk",
                                            tag="dpk", bufs=2)
                        for hh in range(2):
                            nc.sync.dma_start(
                                dpk[:, hh * 4:hh * 4 + 4],
                                rawA[hh][D:D + 1, :]
                                .rearrange("a (p f) -> a p f", p=128))
                        rpk = nrm_pool.tile([128, 8], BF16, name="rpk",
                                            tag="rpk", bufs=2)
                        with nc.allow_low_precision(
                                reason="softmax denom reciprocal in bf16"):
                            nc.vector.reciprocal(rpk[:], dpk[:])
                        rdr = [nrm_pool.tile([1, 512], BF16,
                                              name=f"rdr{hh}", tag="rdr",
                                              bufs=4,
                                              padded_shape=[128, 512])
                               for hh in range(2)]
                        for hh in range(2):
                            nc.sync.dma_start(
                                rdr[hh][:]
                                .rearrange("a (p f) -> a p f", p=128),
                                rpk[:, hh * 4:hh * 4 + 4])
                        for hh in range(2):
                            bcast = nrm_pool.tile([D, 512], BF16,
                                                  name=f"bcast{hh}",
                                                  tag="bcast", bufs=4,
                                                  padded_shape=[128, 512])
                            nc.gpsimd.partition_broadcast(bcast[:],
                                                          rdr[hh][:])
                            rawN = nrm_pool.tile([D, 512], BF16,
                                                 name=f"rawN{hh}", tag="rawN",
                                                 bufs=4,
                                                 padded_shape=[128, 512])
                            nc.vector.tensor_mul(rawN[:], rawA[hh][0:D, :],
                                                 bcast[:])
                            nc.gpsimd.dma_start(
                                a2a_in[hp][2 * qc:2 * qc + 2,
                                           hh * D:hh * D + D, :]
                                .transpose([1, 0, 2]),
                                rawN[:].rearrange("p (j q) -> p j q", j=2))
                    emit_a2a(hp)
                unpack(0)

            # ---- phase 3: o^T projection (g-major accumulation) ----
            with (
                tc.tile_pool(name="o_ps", bufs=1, space="PSUM") as o_ps,
                tc.tile_pool(name="o_sb", bufs=2) as o_sb_pool,
            ):
                unpack(1)
                po = {(p, sb, nf): o_ps.tile([128, 512], F32,
                                             name=f"po{p}{sb}{nf}", tag="po",
                                             bufs=8)
                      for p in range(2) for sb in range(2) for nf in range(2)}
                # group 0 contributions run while the second AllToAll
                # is still in flight
                for i in range(N_CORES):
                    kc = 2 * i
                    for sb in range(2):
                        lhsT = att_g[0][:, i, sb * 128:(sb + 1) * 128]
                        for p in range(2):
                            for nf in range(2):
                                nc.tensor.matmul(
                                    po[(p, sb, nf)][:], lhsT,
                                    wo_sb[:, kc,
                                          (p * 2 + nf) * 512:
                                          (p * 2 + nf + 1) * 512],
                                    start=(i == 0), stop=False)
                # group 1: finish one seq-block at a time so its output
                # copies/DMA overlap the other block's matmuls
                for sb in range(2):
                    for i in range(N_CORES):
                        kc = 2 * i + 1
                        lhsT = att_g[1][:, i, sb * 128:(sb + 1) * 128]
                        for p in range(2):
                            for nf in range(2):
                                nc.tensor.matmul(
                                    po[(p, sb, nf)][:], lhsT,
                                    wo_sb[:, kc,
                                          (p * 2 + nf) * 512:
                                          (p * 2 + nf + 1) * 512],
                                    start=False, stop=(i == N_CORES - 1))
                    for p in range(2):
                        o_out = o_sb_pool.tile([128, 1024], F32, name="o_out",
                                               tag="oo", bufs=4)
                        nc.vector.tensor_copy(o_out[:, 0:512],
                                              po[(p, sb, 0)][:])
                        nc.scalar.activation(
                            o_out[:, 512:1024], po[(p, sb, 1)][:],
                            mybir.ActivationFunctionType.Copy)
                        nc.sync.dma_start(
                            out[sb * 128:(sb + 1) * 128,
                                p * 1024:(p + 1) * 1024],
                            o_out[:])
            wo_ctx.__exit__(None, None, None)

    nc.compile()
    return nc


def _emit_av(nc, acc, vp_sb, pend, kbmax):
    pt, s0, kb = pend
    for hh in range(2):
        nc.tensor.matmul(
            acc[hh][:, s0:512],
            vp_sb[:, kb, :],
            pt[:, hh, s0:512],
            start=(kb == 0), stop=(kb == kbmax - 1),
            skip_group_check=(kb > 0))


def _host_inputs(x, w_qkv, w_o):
    """Build the 8 per-core input maps (host-side staging)."""
    bf = ml_dtypes.bfloat16
    x2 = x.reshape(S, H)
    # xt_b[p, kc, s] = x[s, kc*128+p]
    xt_b = np.ascontiguousarray(
        x2.astype(bf).T.reshape(KC, 128, S).transpose(1, 0, 2))
    wo_t = np.ascontiguousarray(w_o.T).astype(bf)                    # [H, H]

    # rope tables (match reference: inv_freq over even dims, outer with t)
    inv_freq = 1.0 / (10000.0 ** (np.arange(0, D, 2, dtype=np.float32) / D))
    t = np.arange(S, dtype=np.float32)
    freqs = np.outer(t, inv_freq)                                    # [S, 32]
    cos = np.cos(freqs).T.astype(bf)                                 # [32, S]
    sin = np.sin(freqs).T.astype(bf)
    cos4 = np.ascontiguousarray(np.tile(cos, (4, 1)))                # [128, S]
    sin4 = np.ascontiguousarray(np.tile(sin, (4, 1)))

    # causal bias for the diagonal 128-block, staged as lhsT so that
    # tri.T @ I == bias with bias[k, q] = 0 if q >= k else -30000
    kk = np.arange(128)
    tri = np.where(kk[:, None] >= kk[None, :], 0.0, -30000.0).astype(bf)
    tri = np.ascontiguousarray(tri)                                  # [128,128]
    ident = np.eye(128, dtype=bf)

    in_maps = []
    for c in range(N_CORES):
        cols = []
        # q first halves, q second halves (head-packed, 32 rows each)
        for half in range(2):
            for h in range(QH):
                g = (c * QH + h) * D + half * 32
                cols.append(w_qkv[g:g + 32, :])
        # k halves
        kbase = NH * D + c * D
        cols.append(w_qkv[kbase:kbase + 32, :])
        cols.append(w_qkv[kbase + 32:kbase + 64, :])
        # v
        vbase = NH * D + KVH * D + c * D
        cols.append(w_qkv[vbase:vbase + D, :])
        wq_c = np.concatenate(cols, axis=0)                          # [F, H]
        # wq_b[p, kc, m] = wq_c[m, kc*128+p]
        wq_b = np.ascontiguousarray(
            wq_c.astype(bf).T.reshape(KC, 128, F).transpose(1, 0, 2))
        in_maps.append({
            "xt": xt_b, "wq": wq_b, "wo": wo_t,
            "cos4": cos4, "sin4": sin4, "tri": tri, "ident": ident,
        })
    return in_maps


def _run(x, w_qkv, w_o, trace=False):
    if "nc" not in _NC_CACHE:
        _NC_CACHE["nc"] = _build_nc()
    nc = _NC_CACHE["nc"]
    in_maps = _host_inputs(x, w_qkv, w_o)
    res = bass_utils.run_bass_kernel_spmd(
        nc, in_maps, core_ids=list(range(N_CORES)), trace=trace)
    out = np.concatenate(
        [res.results[c]["out"] for c in range(N_CORES)], axis=0)
    return out.reshape(1, S, H).astype(np.float32), res


def kernel(x, w_qkv, w_o):
    out, _ = _run(np.asarray(x), np.asarray(w_qkv), np.asarray(w_o))
    return out


# revision 23
# speedup vs baseline: 1.1214x; 1.1214x over previous
"""Tensor-parallel GQA attention layer for one TRN2 chip (8 NeuronCores).

Problem (hardcoded): x [1, 2048, 2048] f32, w_qkv [3072, 2048] f32,
w_o [2048, 2048] f32; NH=32 q heads, KVH=8 kv heads, D=64, causal, RoPE
(non-interleaved half rotation), GQA group = 4.

Sharding: core c owns kv head c and q heads [4c, 4c+4). Each core:
  1. qkv^T projection in bf16 (fp8 here fails the 2e-2 error budget),
     x^T/w_qkv staged host-side as [128, kc, *] chunks, fed from two DMA
     queues
  2. RoPE on q/k (DVE), outputs fp8e4 in per-seq-half [32, 2, 1024]
     DoubleRow layout
  3. causal flash attention, head-pair loop: scores via fp8 DoubleRow
     (D=64 as 2x32), shared K/V weight loads across the pair, one exp per
     (kb, qc) covering both heads, V'=[V|ones] accumulates the softmax
     denominator; sender-side normalization: packed [128,8] reciprocal +
     gpsimd partition_broadcast (keeps PE/PSUM out of the chain)
  4. AllToAll per head pair (normalized bf16 payload); each group is
     unpacked by a single transposed DMA into its own tile so the o-proj
     dependency is exact (multi-writer unpacking raced)
  5. o^T projection, g-major accumulation: the first group's 64 matmuls
     run at full clock while the second AllToAll is in flight
Host concatenates the 8 [256, 2048] f32 outputs.
"""

import math

import numpy as np
import ml_dtypes

import concourse.bass as bass
import concourse.bacc as bacc
import concourse.tile as tile
import concourse.mybir as mybir
from concourse import bass_utils
from concourse.tile_legalize import tile_legalize as _tile_legalize_orig

N_CORES = 8
S = 2048           # sequence length
H = 2048           # hidden size
NH, KVH, D = 32, 8, 64
QH = NH // N_CORES # q heads per core = 4
F = QH * D + 2 * D # w_qkv rows per core = 384
SB = S // N_CORES  # seq rows per core = 256
KC = H // 128      # 16 contraction chunks of 128
KC2 = KC // 2      # 8 DoubleRow chunks of 256

BF16 = mybir.dt.bfloat16
FP8 = mybir.dt.float8e4
F32 = mybir.dt.float32
DR = mybir.MatmulPerfMode.DoubleRow
SCALE = 1.0 / math.sqrt(D)

_NC_CACHE = {}

# PE-stream instruction types that do not disturb loaded PE weights (they
# either run on other engines or are waits). Anything else resets the
# "weights currently loaded" tracking.
_LDW_SAFE = {
    "InstMatmult", "InstEventSemaphore", "InstTensorCopy", "InstTensorTensor",
    "InstActivation", "InstDMACopy", "InstReciprocal", "InstMemset",
    "InstCollectiveCompute", "InstTensorScalarPtr", "InstCustomDveAnt",
    "InstCopyPredicated", "InstTensorReduce", "InstIota",
}


def _tile_legalize_dedup_ldw(ordered_instructions_by_block, nc):
    """tile_legalize, then drop InstLdweights that reload the exact weights
    already resident in the PE array (consecutive duplicates with only
    non-PE-array-disturbing instructions in between). All matmul weights in
    this kernel are write-once tiles, so a duplicate load is always
    redundant."""
    out = _tile_legalize_orig(ordered_instructions_by_block, nc)
    for bb, insts in out.items():
        last_key = None
        kept = []
        for ins in insts:
            tn = type(ins).__name__
            if tn == "InstLdweights":
                key = repr(ins.ins[0])
                if key == last_key:
                    continue
                last_key = key
            elif tn in _LDW_SAFE:
                pass
            else:
                last_key = None
            kept.append(ins)
        out[bb] = kept
    return out


if getattr(tile.tile_legalize, "__name__", "") != "_tile_legalize_dedup_ldw":
    tile.tile_legalize = _tile_legalize_dedup_ldw


def _build_nc():
    nc = bacc.Bacc("TRN2", target_bir_lowering=False, debug=False,
                   num_devices=N_CORES)

    xt = nc.dram_tensor("xt", [128, KC, S], BF16, kind="ExternalInput")
    wq = nc.dram_tensor("wq", [128, KC, F], BF16, kind="ExternalInput")
    wo = nc.dram_tensor("wo", [H, H], BF16, kind="ExternalInput")
    cos4 = nc.dram_tensor("cos4", [128, S], BF16, kind="ExternalInput")
    sin4 = nc.dram_tensor("sin4", [128, S], BF16, kind="ExternalInput")
    tri = nc.dram_tensor("tri", [128, 128], BF16, kind="ExternalInput")
    ident = nc.dram_tensor("ident", [128, 128], BF16, kind="ExternalInput")
    out = nc.dram_tensor("out", [SB, H], F32, kind="ExternalOutput")

    with tile.TileContext(nc) as tc:
        with (
            tc.tile_pool(name="const", bufs=1) as const_pool,
            tc.tile_pool(name="persist", bufs=1) as persist,
            tc.tile_pool(name="pt_pool", bufs=16) as pt_pool,
            tc.tile_pool(name="qkvh_pool", bufs=1) as qkvh_pool,
            tc.tile_pool(name="rope_tmp", bufs=1) as rope_tmp,
            tc.tile_pool(name="dram", bufs=1, space="DRAM") as dram,
        ):
            # ---- persistent tiles ----
            # per-pair, per-seq-half Q^T (fp8, DoubleRow [32, 2(j), 2(h),
            # 1024]) so one DR matmul produces both heads' scores; K^T per
            # seq-half
            qt_sb = [[persist.tile([32, 2, 2, 1024], FP8,
                                   name=f"qtp{hp}_{hf}")
                      for hf in range(2)] for hp in range(2)]
            kt_sb = [persist.tile([32, 2, 1024], FP8, name=f"kt{hf}")
                     for hf in range(2)]
            vp_sb = persist.tile([128, KC, D + 1], BF16)
            # per-a2a-group o-proj inputs, each written by exactly ONE DMA:
            # att_g[g][:, i, :] = chunk kc=2i+g (heads 4i+2g, 4i+2g+1)
            att_g = [persist.tile([128, N_CORES, SB], BF16, name=f"attg{g}")
                     for g in range(2)]

            tri_sb = const_pool.tile([128, 128], BF16)
            id_sb = const_pool.tile([128, 128], BF16)
            cos_sb = const_pool.tile([128, S], BF16)
            sin_sb = const_pool.tile([128, S], BF16)

            def emit_s_exp(hp, qc, kb, sc_pool):
                """Scores (fp8 DoubleRow, both heads of the pair into one
                PSUM tile) + causal bias + one exp. Returns (pt, s0, kb)."""
                q0 = qc * 512
                k0 = kb * 128
                s0 = max(0, k0 - q0)
                sc = sc_pool.tile([128, 2, 512], F32, name="sc", tag="sc")
                khf, kof = kb // 8, k0 % 1024
                qhf, qof = qc // 2, (q0 % 1024) + s0
                kslc = kt_sb[khf][:, :, kof:kof + 128]
                for hh in range(2):
                    nc.tensor.matmul(
                        sc[:, hh, s0:512], kslc,
                        qt_sb[hp][qhf][:, :, hh, qof:qof + 512 - s0],
                        start=True, stop=True, perf_mode=DR)
                if kb >= 4 * qc:  # diagonal block: causal bias
                    for hh in range(2):
                        nc.tensor.matmul(
                            sc[:, hh, s0:s0 + 128], tri_sb[:], id_sb[:],
                            start=False, stop=True, skip_group_check=True)
                pt = pt_pool.tile([128, 2, 512], BF16, name="pt", tag="pt")
                nc.scalar.activation(
                    pt[:, :, s0:512], sc[:, :, s0:512],
                    mybir.ActivationFunctionType.Exp, scale=SCALE)
                return (pt, s0, kb)

            prestash = {}

            # ---- phase 0/1: input DMA issue + qkv^T projection + RoPE ----
            with (
                tc.tile_pool(name="xt_pool", bufs=1) as xt_pool,
                tc.tile_pool(name="pp_ps", bufs=1, space="PSUM") as pp_ps,
                tc.tile_pool(name="tp_ps", bufs=2, space="PSUM") as tp_ps,
            ):
                xt_sb = xt_pool.tile([128, KC, S], BF16)
                wq_sb = xt_pool.tile([128, KC, F], BF16)
                nc.sync.dma_start(wq_sb[:, 0:1, :], wq[:, 0:1, :])
                nc.sync.dma_start(xt_sb[:, 0, 0:512], xt[:, 0, 0:512])
                nc.sync.dma_start(xt_sb[:, 0, 512:S], xt[:, 0, 512:S])
                nc.scalar.dma_start(xt_sb[:, 1:2, :], xt[:, 1:2, :])
                nc.sync.dma_start(wq_sb[:, 1:KC, :], wq[:, 1:KC, :])
                for kc in range(2, KC):
                    eng = nc.sync if kc % 2 == 0 else nc.scalar
                    eng.dma_start(xt_sb[:, kc:kc + 1, :], xt[:, kc:kc + 1, :])
                    if kc == 5:
                        nc.scalar.dma_start(cos_sb[:], cos4[:])
                        nc.sync.dma_start(sin_sb[:], sin4[:])
                nc.sync.dma_start(tri_sb[:], tri[:])
                nc.sync.dma_start(id_sb[:], ident[:])
                nc.gpsimd.memset(vp_sb[:, :, D:D + 1], 1.0)
                # preload the Exp activation table while the PE works
                warm = rope_tmp.tile([1, 1], BF16, name="warm",
                                     padded_shape=[128, 1])
                nc.gpsimd.memset(warm[:], 0.0)
                nc.scalar.activation(warm[:], warm[:],
                                     mybir.ActivationFunctionType.Exp)

                def emit_qkv_m(half, m, qkvh):
                    pp = [pp_ps.tile([128, 512], F32, name=f"pp{half}{m}{nq}",
                                     tag="pp", bufs=6) for nq in range(2)]
                    for kc in range(KC):
                        lhsT = wq_sb[:, kc, m * 128:(m + 1) * 128]
                        for nq in range(2):
                            c0 = half * 1024 + nq * 512
                            nc.tensor.matmul(
                                pp[nq][:], lhsT,
                                xt_sb[:, kc, c0:c0 + 512],
                                start=(kc == 0), stop=(kc == KC - 1))
                    for nq in range(2):
                        dst = qkvh[:, m, nq * 512:(nq + 1) * 512]
                        if m == 2:
                            nc.vector.tensor_copy(dst, pp[nq][:])
                        else:
                            nc.scalar.activation(
                                dst, pp[nq][:],
                                mybir.ActivationFunctionType.Copy)

                def emit_kv_extras(half, qkvh):
                    hs = slice(half * 1024, (half + 1) * 1024)
                    k1 = qkvh[0:32, 2, :]
                    k2_t = rope_tmp.tile([32, 1024], BF16,
                                         name=f"k2_{half}", tag="k2",
                                         bufs=2, padded_shape=[128, 1024])
                    nc.gpsimd.dma_start(k2_t[:], qkvh[32:64, 2, :])
                    k2 = k2_t[:]
                    u1 = rope_tmp.tile([32, 1024], BF16, name="u1",
                                       tag="kt", bufs=2,
                                       padded_shape=[128, 1024])
                    u2 = rope_tmp.tile([32, 1024], BF16, name="u2",
                                       tag="kt", bufs=2,
                                       padded_shape=[128, 1024])
                    kA = rope_tmp.tile([32, 1024], FP8, name="kA",
                                       tag="kf", bufs=4,
                                       padded_shape=[128, 1024])
                    kB = rope_tmp.tile([32, 1024], FP8, name="kB",
                                       tag="kf", bufs=4,
                                       padded_shape=[128, 1024])
                    c32 = cos_sb[0:32, hs]
                    s32 = sin_sb[0:32, hs]
                    with nc.allow_low_precision(reason="fp8 attention"):
                        nc.vector.tensor_mul(u1[:], k1, c32)
                        nc.vector.tensor_mul(u2[:], k2, s32)
                        nc.vector.tensor_sub(kA[:], u1[:], u2[:])
                        nc.vector.tensor_mul(u1[:], k2, c32)
                        nc.vector.tensor_mul(u2[:], k1, s32)
                        nc.vector.tensor_add(kB[:], u1[:], u2[:])
                    nc.sync.dma_start(kt_sb[half][:, 0, :], kA[:])
                    nc.gpsimd.dma_start(kt_sb[half][:, 1, :], kB[:])
                    vt_bf = rope_tmp.tile([64, 1024], BF16,
                                          name=f"vt{half}", tag="vt",
                                          bufs=2, padded_shape=[128, 1024])
                    nc.gpsimd.dma_start(vt_bf[:], qkvh[64:128, 2, :])
                    for kb in range(half * 8, half * 8 + 8):
                        tp = tp_ps.tile([128, D], BF16, name="tp")
                        nc.tensor.transpose(
                            tp[:],
                            vt_bf[:, (kb % 8) * 128:(kb % 8) * 128 + 128],
                            id_sb[0:64, 0:64])
                        nc.vector.tensor_copy(vp_sb[:, kb, 0:D], tp[:])

                def emit_qrope(half, qkvh):
                    hs = slice(half * 1024, (half + 1) * 1024)
                    q1 = qkvh[:, 0, :]
                    q2 = qkvh[:, 1, :]
                    t1 = rope_tmp.tile([128, 1024], BF16, name="t1", tag="qt",
                                       bufs=2)
                    t2 = rope_tmp.tile([128, 1024], BF16, name="t2", tag="qt",
                                       bufs=2)
                    qA = rope_tmp.tile([128, 1024], FP8, name="qA", tag="qf",
                                       bufs=4)
                    qB = rope_tmp.tile([128, 1024], FP8, name="qB", tag="qf",
                                       bufs=4)
                    with nc.allow_low_precision(reason="fp8 attention"):
                        nc.vector.tensor_mul(t1[:], q1, cos_sb[:, hs])
                        nc.vector.tensor_mul(t2[:], q2, sin_sb[:, hs])
                        nc.vector.tensor_sub(qA[:], t1[:], t2[:])
                        nc.vector.tensor_mul(t1[:], q2, cos_sb[:, hs])
                        nc.vector.tensor_mul(t2[:], q1, sin_sb[:, hs])
                        nc.vector.tensor_add(qB[:], t1[:], t2[:])
                    for h in range(QH):
                        sl = slice(h * 32, (h + 1) * 32)
                        hp, hh = h // 2, h % 2
                        nc.sync.dma_start(qt_sb[hp][half][:, 0, hh, :],
                                          qA[sl, :])
                        nc.gpsimd.dma_start(qt_sb[hp][half][:, 1, hh, :],
                                            qB[sl, :])

                qkvh0 = qkvh_pool.tile([128, 3, 1024], BF16, name="qkvh0",
                                       tag="qkvh", bufs=2)
                emit_qkv_m(0, 2, qkvh0)
                emit_kv_extras(0, qkvh0)
                emit_qkv_m(0, 0, qkvh0)
                emit_qkv_m(0, 1, qkvh0)
                emit_qrope(0, qkvh0)

                # half 1 interleaved with prestashed scores+exp of pair 0,
                # q-chunks 0/1 (they depend only on half-0 outputs); their
                # AVs run in the attention phase when PSUM frees up.
                qkvh1 = qkvh_pool.tile([128, 3, 1024], BF16, name="qkvh1",
                                       tag="qkvh", bufs=2)
                emit_qkv_m(1, 2, qkvh1)
                emit_kv_extras(1, qkvh1)
                emit_qkv_m(1, 0, qkvh1)
                emit_qkv_m(1, 1, qkvh1)

            # half-1 q-rope emitted after the phase-1 pool barrier: it runs
            # on DVE concurrently with the first attention q-chunks (which
            # depend only on half-0 outputs)
            emit_qrope(1, qkvh1)

            # wo occupies the SBUF space freed by xt; its 8 MB DMA streams
            # during attention and must land before phase 3.
            wo_ctx = tc.tile_pool(name="wo_pool", bufs=1)
            wo_pool = wo_ctx.__enter__()
            wo_sb = wo_pool.tile([128, KC, H], BF16)
            nc.gpsimd.dma_start(
                wo_sb[:], wo[:].rearrange("(kc p) f -> p kc f", p=128))

            # ---- phase 2: attention (head pairs) ----
            # a2a group g carries local heads {2g, 2g+1}; payload is the
            # normalized bf16 attention output, rows h*64..h*64+64.
            a2a_in = [dram.tile([N_CORES, 128, SB], BF16, name=f"a2ai{g}")
                      for g in range(2)]
            a2a_out = [dram.tile([N_CORES, 128, SB], BF16, name=f"a2ao{g}")
                       for g in range(2)]

            def emit_a2a(g):
                nc.gpsimd.collective_compute(
                    "AllToAll",
                    mybir.AluOpType.bypass,
                    replica_groups=[list(range(N_CORES))],
                    ins=[a2a_in[g][:]],
                    outs=[a2a_out[g][:]],
                )

            def unpack(g):
                nc.sync.dma_start(att_g[g][:],
                                  a2a_out[g][:].transpose([1, 0, 2]))

            with (
                tc.tile_pool(name="sc_ps", bufs=2, space="PSUM") as sc_ps,
                tc.tile_pool(name="acc_ps", bufs=4, space="PSUM") as acc_ps,
                tc.tile_pool(name="nrm_pool", bufs=1) as nrm_pool,
            ):
                for hp in range(2):
                    for qc in range(4):
                        acc = [acc_ps.tile([D + 1, 512], F32,
                                           name=f"acc{hp}{qc}{hh}", tag="acc")
                               for hh in range(2)]
                        kbmax = 4 * qc + 4
                        pend = None  # (pt, s0, kb) awaiting AV emission
                        for kb in range(kbmax):
                            if (hp, qc, kb) in prestash:
                                nxt = prestash[(hp, qc, kb)]
                            else:
                                nxt = emit_s_exp(hp, qc, kb, sc_ps)
                            if pend is not None:
                                _emit_av(nc, acc, vp_sb, pend, kbmax)
                            pend = nxt
                        _emit_av(nc, acc, vp_sb, pend, kbmax)

                        # ---- sender-side normalize + ship ----
                        rawA = [nrm_pool.tile([D + 1, 512], BF16,
                                              name=f"rawA{hh}", tag="rawA",
                                              bufs=4, padded_shape=[128, 512])
                                for hh in range(2)]
                        for hh in range(2):
                            nc.vector.tensor_copy(rawA[hh][:], acc[hh][:])
                        dpk = nrm_pool.tile([128, 8], BF16, name="dpk",
                                            tag="dpk", bufs=2)
                        for hh in range(2):
                            nc.sync.dma_start(
                                dpk[:, hh * 4:hh * 4 + 4],
                                rawA[hh][D:D + 1, :]
                                .rearrange("a (p f) -> a p f", p=128))
                        rpk = nrm_pool.tile([128, 8], BF16, name="rpk",
                                            tag="rpk", bufs=2)
                        with nc.allow_low_precision(
                                reason="softmax denom reciprocal in bf16"):
                            nc.vector.reciprocal(rpk[:], dpk[:])
                        rdr = [nrm_pool.tile([1, 512], BF16,
                                              name=f"rdr{hh}", tag="rdr",
                                              bufs=4,
                                              padded_shape=[128, 512])
                               for hh in range(2)]
                        for hh in range(2):
                            nc.sync.dma_start(
                                rdr[hh][:]
                                .rearrange("a (p f) -> a p f", p=128),
                                rpk[:, hh * 4:hh * 4 + 4])
                        for hh in range(2):
                            bcast = nrm_pool.tile([D, 512], BF16,
                                                  name=f"bcast{hh}",
                                                  tag="bcast", bufs=4,
                                                  padded_shape=[128, 512])
                            nc.gpsimd.partition_broadcast(bcast[:],
                                                          rdr[hh][:])
                            rawN = nrm_pool.tile([D, 512], BF16,
                                                 name=f"rawN{hh}", tag="rawN",
                                                 bufs=4,
                                                 padded_shape=[128, 512])
                            nc.vector.tensor_mul(rawN[:], rawA[hh][0:D, :],
                                                 bcast[:])
                            nc.gpsimd.dma_start(
                                a2a_in[hp][2 * qc:2 * qc + 2,
                                           hh * D:hh * D + D, :]
                                .transpose([1, 0, 2]),
                                rawN[:].rearrange("p (j q) -> p j q", j=2))
                    emit_a2a(hp)
                unpack(0)

            # ---- phase 3: o^T projection (g-major accumulation) ----
            with (
                tc.tile_pool(name="o_ps", bufs=1, space="PSUM") as o_ps,
                tc.tile_pool(name="o_sb", bufs=2) as o_sb_pool,
            ):
                unpack(1)
                po = {(p, sb, nf): o_ps.tile([128, 512], F32,
                                             name=f"po{p}{sb}{nf}", tag="po",
                                             bufs=8)
                      for p in range(2) for sb in range(2) for nf in range(2)}
                # group 0 contributions run while the second AllToAll
                # is still in flight
                for i in range(N_CORES):
                    kc = 2 * i
                    for sb in range(2):
                        lhsT = att_g[0][:, i, sb * 128:(sb + 1) * 128]
                        for p in range(2):
                            for nf in range(2):
                                nc.tensor.matmul(
                                    po[(p, sb, nf)][:], lhsT,
                                    wo_sb[:, kc,
                                          (p * 2 + nf) * 512:
                                          (p * 2 + nf + 1) * 512],
                                    start=(i == 0), stop=False)
                # group 1: finish one seq-block at a time so its output
                # copies/DMA overlap the other block's matmuls
                for sb in range(2):
                    for i in range(N_CORES):
                        kc = 2 * i + 1
                        lhsT = att_g[1][:, i, sb * 128:(sb + 1) * 128]
                        for p in range(2):
                            for nf in range(2):
                                nc.tensor.matmul(
                                    po[(p, sb, nf)][:], lhsT,
                                    wo_sb[:, kc,
                                          (p * 2 + nf) * 512:
                                          (p * 2 + nf + 1) * 512],
                                    start=False, stop=(i == N_CORES - 1))
                    for p in range(2):
                        o_out = o_sb_pool.tile([128, 1024], F32, name="o_out",
                                               tag="oo", bufs=4)
                        nc.vector.tensor_copy(o_out[:, 0:512],
                                              po[(p, sb, 0)][:])
                        nc.scalar.activation(
                            o_out[:, 512:1024], po[(p, sb, 1)][:],
                            mybir.ActivationFunctionType.Copy)
                        nc.sync.dma_start(
                            out[sb * 128:(sb + 1) * 128,
                                p * 1024:(p + 1) * 1024],
                            o_out[:])
            wo_ctx.__exit__(None, None, None)

    nc.compile()
    return nc


def _emit_av(nc, acc, vp_sb, pend, kbmax):
    pt, s0, kb = pend
    for hh in range(2):
        nc.tensor.matmul(
            acc[hh][:, s0:512],
            vp_sb[:, kb, :],
            pt[:, hh, s0:512],
            start=(kb == 0), stop=(kb == kbmax - 1),
            skip_group_check=(kb > 0))


def _host_inputs(x, w_qkv, w_o):
    """Build the 8 per-core input maps (host-side staging)."""
    bf = ml_dtypes.bfloat16
    x2 = x.reshape(S, H)
    # xt_b[p, kc, s] = x[s, kc*128+p]
    xt_b = np.ascontiguousarray(
        x2.astype(bf).T.reshape(KC, 128, S).transpose(1, 0, 2))
    wo_t = np.ascontiguousarray(w_o.T).astype(bf)                    # [H, H]

    # rope tables (match reference: inv_freq over even dims, outer with t)
    inv_freq = 1.0 / (10000.0 ** (np.arange(0, D, 2, dtype=np.float32) / D))
    t = np.arange(S, dtype=np.float32)
    freqs = np.outer(t, inv_freq)                                    # [S, 32]
    cos = np.cos(freqs).T.astype(bf)                                 # [32, S]
    sin = np.sin(freqs).T.astype(bf)
    cos4 = np.ascontiguousarray(np.tile(cos, (4, 1)))                # [128, S]
    sin4 = np.ascontiguousarray(np.tile(sin, (4, 1)))

    # causal bias for the diagonal 128-block, staged as lhsT so that
    # tri.T @ I == bias with bias[k, q] = 0 if q >= k else -30000
    kk = np.arange(128)
    tri = np.where(kk[:, None] >= kk[None, :], 0.0, -30000.0).astype(bf)
    tri = np.ascontiguousarray(tri)                                  # [128,128]
    ident = np.eye(128, dtype=bf)

    in_maps = []
    for c in range(N_CORES):
        cols = []
        # q first halves, q second halves (head-packed, 32 rows each)
        for half in range(2):
            for h in range(QH):
                g = (c * QH + h) * D + half * 32
                cols.append(w_qkv[g:g + 32, :])
        # k halves
        kbase = NH * D + c * D
        cols.append(w_qkv[kbase:kbase + 32, :])
        cols.append(w_qkv[kbase + 32:kbase + 64, :])
        # v
        vbase = NH * D + KVH * D + c * D
        cols.append(w_qkv[vbase:vbase + D, :])
        wq_c = np.concatenate(cols, axis=0)                          # [F, H]
        # wq_b[p, kc, m] = wq_c[m, kc*128+p]
        wq_b = np.ascontiguousarray(
            wq_c.astype(bf).T.reshape(KC, 128, F).transpose(1, 0, 2))
        in_maps.append({
            "xt": xt_b, "wq": wq_b, "wo": wo_t,
            "cos4": cos4, "sin4": sin4, "tri": tri, "ident": ident,
        })
    return in_maps


def _run(x, w_qkv, w_o, trace=False):
    if "nc" not in _NC_CACHE:
        _NC_CACHE["nc"] = _build_nc()
    nc = _NC_CACHE["nc"]
    in_maps = _host_inputs(x, w_qkv, w_o)
    res = bass_utils.run_bass_kernel_spmd(
        nc, in_maps, core_ids=list(range(N_CORES)), trace=trace)
    out = np.concatenate(
        [res.results[c]["out"] for c in range(N_CORES)], axis=0)
    return out.reshape(1, S, H).astype(np.float32), res


def kernel(x, w_qkv, w_o):
    out, _ = _run(np.asarray(x), np.asarray(w_qkv), np.asarray(w_o))
    return out
